# revision 1
# baseline (speedup 1.0000x reference)
"""Trainium2 Bass kernel for BackprojectDepth.

out[b, i, y*W+x] = depth[b, 0, y, x] * (K[b,i,0]*(x+dx[b]) + K[b,i,1]*(y+dy[b]) + K[b,i,2])   for i in 0..2
out[b, 3, :]    = 1.0

Sharding: pure data parallel over batch (32 batches -> 4 per core on 8 cores).

Per-core device program: for each (batch, row-tile) the affine term
lin[p, m] = A*m + (B*(t*128+p) + A*dx + B*dy + C) is computed on the scalar
(ACT) engine from a constant x-ramp tile with per-partition scale/bias
vectors (host-precomputed from inv_K/dxy), then multiplied elementwise with
the depth tile on the vector engine, and DMA'd out.  The ones-plane is
written from a constant SBUF tile.
"""

import numpy as np

import concourse.bass as bass
import concourse.tile as tile
from concourse import bacc, mybir
from concourse.bass_utils import run_bass_kernel_spmd

N_CORES = 8
B, H, W = 32, 512, 1024
HW = H * W
BPC = B // N_CORES          # batches per core
TPB = H // 128              # row-tiles per batch (partition dim = 128 rows)

F32 = mybir.dt.float32

_TRACE = False              # test.py may flip this for profiling
_LAST_RESULTS = None        # BassKernelResults from the last run (for test.py)

_nc_cache = None


def _build():
    """Build + compile the per-core Bass program (SPMD: same NEFF, 8 cores)."""
    nc = bacc.Bacc(
        "TRN2",
        target_bir_lowering=False,
        debug=False,
        enable_asserts=False,
        num_devices=N_CORES,
    )

    depth_d = nc.dram_tensor("depth", [BPC, H, W], F32, kind="ExternalInput")
    xg_d = nc.dram_tensor("xg", [128, W], F32, kind="ExternalInput")
    scale_d = nc.dram_tensor("scale", [128, BPC * 3], F32, kind="ExternalInput")
    bias_d = nc.dram_tensor("bias", [128, BPC * 3 * TPB], F32, kind="ExternalInput")
    out_d = nc.dram_tensor("out", [BPC, 4, HW], F32, kind="ExternalOutput")

    with tile.TileContext(nc) as tc:
        with (
            tc.tile_pool(name="const", bufs=1) as cpool,
            tc.tile_pool(name="dpool", bufs=4) as dpool,
            tc.tile_pool(name="lpool", bufs=6) as lpool,
            tc.tile_pool(name="opool", bufs=6) as opool,
        ):
            xg_t = cpool.tile([128, W], F32)
            nc.sync.dma_start(xg_t[:], xg_d.ap())
            sc_t = cpool.tile([128, BPC * 3], F32)
            nc.sync.dma_start(sc_t[:], scale_d.ap())
            bi_t = cpool.tile([128, BPC * 3 * TPB], F32)
            nc.sync.dma_start(bi_t[:], bias_d.ap())
            ones_t = cpool.tile([128, HW // 128], F32)
            nc.vector.memset(ones_t[:], 1.0)

            # out[b, i, t*131072 + p*1024 + m]  <->  [b, i, t, p, m]
            out_ap = out_d.ap().rearrange("b i (t p m) -> b i t p m", t=TPB, p=128)
            ones_ap = out_d.ap().rearrange("b i (p m) -> b i p m", p=128)
            depth_ap = depth_d.ap().rearrange("b (t p) m -> b t p m", p=128)

            for b in range(BPC):
                nc.sync.dma_start(ones_ap[b, 3], ones_t[:])
                for t in range(TPB):
                    d_t = dpool.tile([128, W], F32)
                    nc.sync.dma_start(d_t[:], depth_ap[b, t])
                    for i in range(3):
                        col = 3 * b + i
                        lin = lpool.tile([128, W], F32)
                        nc.scalar.activation(
                            lin[:],
                            xg_t[:],
                            mybir.ActivationFunctionType.Identity,
                            bias=bi_t[:, col * TPB + t : col * TPB + t + 1],
                            scale=sc_t[:, col : col + 1],
                        )
                        o_t = opool.tile([128, W], F32)
                        nc.vector.tensor_mul(o_t[:], lin[:], d_t[:])
                        nc.sync.dma_start(out_ap[b, i, t], o_t[:])

    nc.compile()
    return nc


def kernel(depth, inv_K, dxy):
    global _nc_cache, _LAST_RESULTS

    depth = np.ascontiguousarray(np.asarray(depth, dtype=np.float32))
    K = np.asarray(inv_K, dtype=np.float64)
    dx = np.asarray(dxy, dtype=np.float64)

    # Per-batch affine coefficients: cam_i = A*x' + B*y' + C with x'=x+dx, y'=y+dy
    A = K[:, :3, 0]                                   # [B, 3]
    Bc = K[:, :3, 1]
    C = K[:, :3, 2]
    const = A * dx[:, None, 0] + Bc * dx[:, None, 1] + C   # [B, 3]

    p = np.arange(128, dtype=np.float64)
    yrow = 128.0 * np.arange(TPB, dtype=np.float64)[:, None] + p[None, :]  # [TPB,128]
    # bias[g, i, t, p] = B*(128t+p) + const
    bias_all = Bc[:, :, None, None] * yrow[None, None] + const[:, :, None, None]

    xg = np.ascontiguousarray(
        np.broadcast_to(np.arange(W, dtype=np.float32), (128, W))
    )

    in_maps = []
    for c in range(N_CORES):
        g0 = c * BPC
        bias_c = np.ascontiguousarray(
            bias_all[g0 : g0 + BPC]                  # [BPC, 3, TPB, 128]
            .reshape(BPC * 3 * TPB, 128)
            .T.astype(np.float32)
        )                                            # [128, BPC*3*TPB]
        scale_c = np.ascontiguousarray(
            np.broadcast_to(
                A[g0 : g0 + BPC].reshape(BPC * 3).astype(np.float32),
                (128, BPC * 3),
            )
        )
        in_maps.append(
            {
                "depth": depth[g0 : g0 + BPC, 0],    # [BPC, H, W]
                "xg": xg,
                "scale": scale_c,
                "bias": bias_c,
            }
        )

    if _nc_cache is None:
        _nc_cache = _build()

    res = run_bass_kernel_spmd(
        _nc_cache, in_maps, core_ids=list(range(N_CORES)), trace=_TRACE
    )
    _LAST_RESULTS = res

    out = np.empty((B, 4, HW), dtype=np.float32)
    for c in range(N_CORES):
        out[c * BPC : (c + 1) * BPC] = res.results[c]["out"]
    return out


# revision 5
# speedup vs baseline: 1.0979x; 1.0979x over previous
"""Trainium2 Bass kernel for BackprojectDepth.

out[b, i, y*W+x] = depth[b, 0, y, x] * (K[b,i,0]*(x+dx[b]) + K[b,i,1]*(y+dy[b]) + K[b,i,2])   for i in 0..2
out[b, 3, :]    = 1.0

Sharding: pure data parallel over batch (32 batches -> 4 per core on 8 cores).

Per-core device program: for each (batch, row-tile) the affine term
lin[p, m] = A*m + (B*(t*128+p) + A*dx + B*dy + C) is computed on the scalar
(ACT) engine from a constant x-ramp tile with per-partition scale/bias
vectors (host-precomputed from inv_K/dxy), then multiplied elementwise with
the depth tile on the vector engine, and DMA'd out.  The ones-plane is
written from a constant SBUF tile.
"""

import numpy as np

import concourse.bass as bass
import concourse.tile as tile
from concourse import bacc, mybir
from concourse.bass_utils import run_bass_kernel_spmd

N_CORES = 8
B, H, W = 32, 512, 1024
HW = H * W
BPC = B // N_CORES          # batches per core
TPB = H // 128              # row-tiles per batch (partition dim = 128 rows)

F32 = mybir.dt.float32

_TRACE = False              # test.py may flip this for profiling
_LAST_RESULTS = None        # BassKernelResults from the last run (for test.py)

_nc_cache = None


def _build():
    """Build + compile the per-core Bass program (SPMD: same NEFF, 8 cores)."""
    nc = bacc.Bacc(
        "TRN2",
        target_bir_lowering=False,
        debug=False,
        enable_asserts=False,
        num_devices=N_CORES,
    )

    depth_d = nc.dram_tensor("depth", [BPC, H, W], F32, kind="ExternalInput")
    scale_d = nc.dram_tensor("scale", [128, BPC * 3], F32, kind="ExternalInput")
    bias_d = nc.dram_tensor("bias", [128, BPC * 3 * TPB], F32, kind="ExternalInput")
    out_d = nc.dram_tensor("out", [BPC, 4, HW], F32, kind="ExternalOutput")

    with tile.TileContext(nc) as tc:
        with (
            tc.tile_pool(name="const", bufs=1) as cpool,
            tc.tile_pool(name="dpool", bufs=8) as dpool,
            tc.tile_pool(name="lpool", bufs=10) as lpool,
            tc.tile_pool(name="opool", bufs=12) as opool,
        ):
            # x-ramp generated on the (otherwise idle) gpsimd engine
            xg_i = cpool.tile([128, W], mybir.dt.int32)
            nc.gpsimd.iota(xg_i[:], pattern=[[1, W]], base=0, channel_multiplier=0)
            xg_t = cpool.tile([128, W], F32)
            nc.gpsimd.tensor_copy(xg_t[:], xg_i[:])
            sc_t = cpool.tile([128, BPC * 3], F32)
            nc.sync.dma_start(sc_t[:], scale_d.ap())
            bi_t = cpool.tile([128, BPC * 3 * TPB], F32)
            nc.sync.dma_start(bi_t[:], bias_d.ap())
            ones_t = cpool.tile([128, HW // 128], F32)
            nc.gpsimd.memset(ones_t[:], 1.0)

            # out[b, i, t*131072 + p*1024 + m]  <->  [b, i, t, p, m]
            out_ap = out_d.ap().rearrange("b i (t p m) -> b i t p m", t=TPB, p=128)
            ones_ap = out_d.ap().rearrange("b i (p m) -> b i p m", p=128)
            depth_ap = depth_d.ap().rearrange("b (t p) m -> b t p m", p=128)

            for b in range(BPC):
                nc.sync.dma_start(ones_ap[b, 3], ones_t[:])
                for t in range(TPB):
                    d_t = dpool.tile([128, W], F32)
                    nc.sync.dma_start(d_t[:], depth_ap[b, t])
                    for i in range(3):
                        col = 3 * b + i
                        lin = lpool.tile([128, W], F32)
                        nc.scalar.activation(
                            lin[:],
                            xg_t[:],
                            mybir.ActivationFunctionType.Identity,
                            bias=bi_t[:, col * TPB + t : col * TPB + t + 1],
                            scale=sc_t[:, col : col + 1],
                        )
                        o_t = opool.tile([128, W], F32)
                        nc.vector.tensor_mul(o_t[:], lin[:], d_t[:])
                        nc.sync.dma_start(out_ap[b, i, t], o_t[:])

    nc.compile()
    return nc


def kernel(depth, inv_K, dxy):
    global _nc_cache, _LAST_RESULTS

    depth = np.ascontiguousarray(np.asarray(depth, dtype=np.float32))
    K = np.asarray(inv_K, dtype=np.float64)
    dx = np.asarray(dxy, dtype=np.float64)

    # Per-batch affine coefficients: cam_i = A*x' + B*y' + C with x'=x+dx, y'=y+dy
    A = K[:, :3, 0]                                   # [B, 3]
    Bc = K[:, :3, 1]
    C = K[:, :3, 2]
    const = A * dx[:, None, 0] + Bc * dx[:, None, 1] + C   # [B, 3]

    p = np.arange(128, dtype=np.float64)
    yrow = 128.0 * np.arange(TPB, dtype=np.float64)[:, None] + p[None, :]  # [TPB,128]
    # bias[g, i, t, p] = B*(128t+p) + const
    bias_all = Bc[:, :, None, None] * yrow[None, None] + const[:, :, None, None]

    in_maps = []
    for c in range(N_CORES):
        g0 = c * BPC
        bias_c = np.ascontiguousarray(
            bias_all[g0 : g0 + BPC]                  # [BPC, 3, TPB, 128]
            .reshape(BPC * 3 * TPB, 128)
            .T.astype(np.float32)
        )                                            # [128, BPC*3*TPB]
        scale_c = np.ascontiguousarray(
            np.broadcast_to(
                A[g0 : g0 + BPC].reshape(BPC * 3).astype(np.float32),
                (128, BPC * 3),
            )
        )
        in_maps.append(
            {
                "depth": depth[g0 : g0 + BPC, 0],    # [BPC, H, W]
                "scale": scale_c,
                "bias": bias_c,
            }
        )

    if _nc_cache is None:
        _nc_cache = _build()

    res = run_bass_kernel_spmd(
        _nc_cache, in_maps, core_ids=list(range(N_CORES)), trace=_TRACE
    )
    _LAST_RESULTS = res

    out = np.empty((B, 4, HW), dtype=np.float32)
    for c in range(N_CORES):
        out[c * BPC : (c + 1) * BPC] = res.results[c]["out"]
    return out


# revision 7
# speedup vs baseline: 1.1717x; 1.0672x over previous
"""Trainium2 Bass kernel for BackprojectDepth.

out[b, i, y*W+x] = depth[b, 0, y, x] * (K[b,i,0]*(x+dx[b]) + K[b,i,1]*(y+dy[b]) + K[b,i,2])   for i in 0..2
out[b, 3, :]    = 1.0

Sharding: pure data parallel over batch (32 batches -> 4 per core on 8 cores).

Per-core device program: for each (batch, row-tile) the affine term
lin[p, m] = A*m + (B*(t*128+p) + A*dx + B*dy + C) is computed on the scalar
(ACT) engine from a constant x-ramp tile with per-partition scale/bias
vectors (host-precomputed from inv_K/dxy), then multiplied elementwise with
the depth tile on the vector engine, and DMA'd out.  The ones-plane is
written from a constant SBUF tile.
"""

import numpy as np

import concourse.bass as bass
import concourse.tile as tile
from concourse import bacc, mybir
from concourse.bass_utils import run_bass_kernel_spmd

N_CORES = 8
B, H, W = 32, 512, 1024
HW = H * W
BPC = B // N_CORES          # batches per core
TPB = H // 128              # row-tiles per batch (partition dim = 128 rows)

F32 = mybir.dt.float32

_TRACE = False              # test.py may flip this for profiling
_LAST_RESULTS = None        # BassKernelResults from the last run (for test.py)

_nc_cache = None


def _build():
    """Build + compile the per-core Bass program (SPMD: same NEFF, 8 cores)."""
    nc = bacc.Bacc(
        "TRN2",
        target_bir_lowering=False,
        debug=False,
        enable_asserts=False,
        num_devices=N_CORES,
    )

    depth_d = nc.dram_tensor("depth", [BPC, H, W], F32, kind="ExternalInput")
    scale_d = nc.dram_tensor("scale", [128, BPC * 3], F32, kind="ExternalInput")
    bias_d = nc.dram_tensor("bias", [128, BPC * 3 * TPB], F32, kind="ExternalInput")
    out_d = nc.dram_tensor("out", [BPC, 4, HW], F32, kind="ExternalOutput")

    with tile.TileContext(nc) as tc:
        with (
            tc.tile_pool(name="const", bufs=1) as cpool,
            tc.tile_pool(name="dpool", bufs=8) as dpool,
            tc.tile_pool(name="lpool", bufs=10) as lpool,
            tc.tile_pool(name="opool", bufs=12) as opool,
        ):
            # x-ramp generated on the (otherwise idle) gpsimd engine
            xg_i = cpool.tile([128, W], mybir.dt.int32)
            nc.gpsimd.iota(xg_i[:], pattern=[[1, W]], base=0, channel_multiplier=0)
            xg_t = cpool.tile([128, W], F32)
            nc.gpsimd.tensor_copy(xg_t[:], xg_i[:])
            sc_t = cpool.tile([128, BPC * 3], F32)
            nc.scalar.dma_start(sc_t[:], scale_d.ap())
            bi_t = cpool.tile([128, BPC * 3 * TPB], F32)
            nc.scalar.dma_start(bi_t[:], bias_d.ap())
            ones_t = cpool.tile([128, HW // 128], F32)
            nc.gpsimd.memset(ones_t[:], 1.0)

            # out[b, i, t*131072 + p*1024 + m]  <->  [b, i, t, p, m]
            out_ap = out_d.ap().rearrange("b i (t p m) -> b i t p m", t=TPB, p=128)
            ones_ap = out_d.ap().rearrange("b i (p m) -> b i p m", p=128)
            depth_ap = depth_d.ap().rearrange("b (t p) m -> b t p m", p=128)

            for b in range(BPC):
                nc.gpsimd.dma_start(ones_ap[b, 3], ones_t[:])
                for t in range(TPB):
                    d_t = dpool.tile([128, W], F32)
                    nc.scalar.dma_start(d_t[:], depth_ap[b, t])
                    for i in range(3):
                        col = 3 * b + i
                        lin = lpool.tile([128, W], F32)
                        nc.scalar.activation(
                            lin[:],
                            xg_t[:],
                            mybir.ActivationFunctionType.Identity,
                            bias=bi_t[:, col * TPB + t : col * TPB + t + 1],
                            scale=sc_t[:, col : col + 1],
                        )
                        o_t = opool.tile([128, W], F32)
                        nc.vector.tensor_mul(o_t[:], lin[:], d_t[:])
                        nc.sync.dma_start(out_ap[b, i, t], o_t[:])

    nc.compile()
    return nc


def kernel(depth, inv_K, dxy):
    global _nc_cache, _LAST_RESULTS

    depth = np.ascontiguousarray(np.asarray(depth, dtype=np.float32))
    K = np.asarray(inv_K, dtype=np.float64)
    dx = np.asarray(dxy, dtype=np.float64)

    # Per-batch affine coefficients: cam_i = A*x' + B*y' + C with x'=x+dx, y'=y+dy
    A = K[:, :3, 0]                                   # [B, 3]
    Bc = K[:, :3, 1]
    C = K[:, :3, 2]
    const = A * dx[:, None, 0] + Bc * dx[:, None, 1] + C   # [B, 3]

    p = np.arange(128, dtype=np.float64)
    yrow = 128.0 * np.arange(TPB, dtype=np.float64)[:, None] + p[None, :]  # [TPB,128]
    # bias[g, i, t, p] = B*(128t+p) + const
    bias_all = Bc[:, :, None, None] * yrow[None, None] + const[:, :, None, None]

    in_maps = []
    for c in range(N_CORES):
        g0 = c * BPC
        bias_c = np.ascontiguousarray(
            bias_all[g0 : g0 + BPC]                  # [BPC, 3, TPB, 128]
            .reshape(BPC * 3 * TPB, 128)
            .T.astype(np.float32)
        )                                            # [128, BPC*3*TPB]
        scale_c = np.ascontiguousarray(
            np.broadcast_to(
                A[g0 : g0 + BPC].reshape(BPC * 3).astype(np.float32),
                (128, BPC * 3),
            )
        )
        in_maps.append(
            {
                "depth": depth[g0 : g0 + BPC, 0],    # [BPC, H, W]
                "scale": scale_c,
                "bias": bias_c,
            }
        )

    if _nc_cache is None:
        _nc_cache = _build()

    res = run_bass_kernel_spmd(
        _nc_cache, in_maps, core_ids=list(range(N_CORES)), trace=_TRACE
    )
    _LAST_RESULTS = res

    out = np.empty((B, 4, HW), dtype=np.float32)
    for c in range(N_CORES):
        out[c * BPC : (c + 1) * BPC] = res.results[c]["out"]
    return out
